# revision 46
# baseline (speedup 1.0000x reference)
"""Trainium2 Bass kernel for nn_CVFRLayer (recurrent attractor scan).

x_{t+1} = (1-dt)*x_t + nl(x_t) @ B' + z_t,   nl(x) = x^2/(gamma+x^2)
  B' = dt*(A@(I-P) + P).T  (P block-diagonal projector, computed host-side O(n^2))
  z_t = noise_t @ (sqrt(dt)*eps*G.T)

Strategy: pure data parallel over 8 NeuronCores, 64 batch rows per core.
State kept in a "folded" layout [128, 1024]: partitions 0-63 hold features
0-1023, partitions 64-127 hold features 1024-2047 for the same 64 batch rows.
The scan matmul runs as column-tiled pairs (tile_position (0,0)/(0,64)) so two
M=64 matmuls occupy the full 128x128 PE array concurrently (measured: the pair
streams in ~216ns, i.e. 2x effective throughput). Noise projections are
computed on-core as full-M=128 matmuls (2 steps per supertile), interleaved
with the scan so the PE stays busy while the serial per-step epilogue runs.
fT (transposed nonlinearity output, the next step's stationary operand) is
produced by single merged xbar-transpose DMAs (cost is per-instruction, not
per-byte), one per state half, split across the two HWDGE queues (SP + ACT).
"""

import math
import sys

if "/opt/trn_rl_repo" not in sys.path:
    sys.path.insert(0, "/opt/trn_rl_repo")

import numpy as np

SIZE = 2048
N_CLASSES = 16
STEPS = 100
DT = 0.03
GAMMA = 0.125
BETA = 1.0
EIG = 1.0
EPSILON = 0.1

N_CORES = 8
BPC = 64  # batch rows per core
HALF = SIZE // 2  # folded free dim
KT = SIZE // 128  # 16 contraction tiles
A_COEF = 1.0 - DT
CHUNKS = [(0, 512), (512, 1024)]  # folded-col chunks per step

# fp8 (e4m3) noise path: host-side pow2 scales; undone in the psum->sbuf copy
SG_SCALE = 8192.0  # max|G| ~0.022 -> x8192 ~ 181 < 240
SN_SCALE = 32.0  # noise ~N(0,1), max ~5.4 -> x32 ~ 173 < 240
NOISE_OUT_SCALE = math.sqrt(DT) * EPSILON / (SG_SCALE * SN_SCALE) / A_COEF

# scan via the reciprocal trick: f = x^2/(g+x^2) = 1 - g/(g+x^2), so
#   f @ B' = colsum(B') - g*(r @ B'),  r = 1/(x^2 + g)
# kernel matmuls rs = r/R_SC (scale applied in the f32->f16 convert) against
# btq = -g*R_SC*B' (pow2 R_SC keeps both f16-normal), adds colsum(B') via the
# noise z, and folds 1/A_COEF into z so the combine is a plain tensor add:
#   w' = x + (z + c)/A_COEF;  x_new = A_COEF*w' + rs@btq
R_SC = 64.0

_cache = {}


def _build(steps):
    import concourse.bacc as bacc
    import concourse.mybir as mybir
    import concourse.tile as tile

    f16 = mybir.dt.float16
    f32 = mybir.dt.float32
    f8 = mybir.dt.float8e4
    AF = mybir.ActivationFunctionType
    OP = mybir.AluOpType
    PM = mybir.MatmulPerfMode

    n_st = steps // 2  # noise supertiles (2 steps each)
    assert steps % 2 == 0

    nc = bacc.Bacc("TRN2", target_bir_lowering=False, debug=False, num_devices=N_CORES)
    x0_d = nc.declare_dram_parameter("x0", [128, HALF], f32, isOutput=False)
    bt_d = nc.declare_dram_parameter("bt", [SIZE, SIZE], f16, isOutput=False)
    gt_d = nc.declare_dram_parameter("gt", [SIZE, SIZE], f8, isOutput=False)
    nt_d = nc.declare_dram_parameter("nt", [SIZE, steps * BPC], f8, isOutput=False)
    cb_d = nc.declare_dram_parameter("cb", [128, SIZE], f32, isOutput=False)
    out_d = nc.declare_dram_parameter("out", [128, HALF], f32, isOutput=True)

    # 3D views with the 128-partition dim first: [(k p) n -> p k n]
    bt_v = bt_d.rearrange("(k p) n -> p k n", p=128)
    gt_v = gt_d.rearrange("(k p) n -> p k n", p=128)
    nt_v = nt_d.rearrange("(k p) n -> p k n", p=128)

    with tile.TileContext(nc) as tc:
        with (
            tc.tile_pool(name="const", bufs=1) as constp,
            tc.tile_pool(name="state", bufs=1) as statep,
            tc.tile_pool(name="f16t", bufs=2) as f16p,
            tc.tile_pool(name="fT", bufs=2) as fTp,
            tc.tile_pool(name="zu", bufs=2) as zup,
            tc.tile_pool(name="zf", bufs=6) as zfp,
            tc.tile_pool(name="nt", bufs=3) as ntp,
            tc.tile_pool(name="scr", bufs=4) as scrp,
            tc.tile_pool(name="sps", bufs=2, space="PSUM") as spsp,
            tc.tile_pool(name="nps", bufs=3, space="PSUM") as npsp,
        ):
            # ---- persistent tiles ----
            bt = constp.tile([128, KT, SIZE], f16, tag="bt")
            gt = constp.tile([128, KT, SIZE], f8, tag="gt")
            cb = constp.tile([128, SIZE], f32, tag="cb")
            x = statep.tile([128, HALF], f32, tag="x")
            w = statep.tile([128, HALF], f32, tag="w")

            # queue order matters (single regular-DMA queue): lead noise tiles
            # first (small, needed immediately), then gt chunks, x0, bt last

            def nl_chunk(src_ap, dst_ap, wd):
                """dst(f16) = 1/(src^2 + gamma) for a [128, wd] slice."""
                s = scrp.tile([128, 512], f32, tag="s")
                d = scrp.tile([128, 512], f32, tag="d")
                r = scrp.tile([128, 512], f32, tag="r")
                nc.scalar.activation(s[:, :wd], src_ap, AF.Square)
                nc.vector.tensor_scalar_add(d[:, :wd], s[:, :wd], GAMMA)
                nc.vector.reciprocal_approx_fast(r[:, :wd], d[:, :wd])
                nc.scalar.activation(dst_ap, r[:, :wd], AF.Copy, scale=1.0 / R_SC)

            def transposes_chunk(f16t, fT_next, ci):
                # per-chunk quarter transposes: chunk ci covers fT k-tiles
                # 4ci..4ci+3 (state half 0) and 8+4ci..8+4ci+3 (half 1),
                # split across the two HWDGE queues
                # both on the sync queue: a dma_start BLOCKS the issuing
                # engine until the DMA completes, and sync runs nothing
                # chain-critical (ACT does: Sq/Copy must not stall)
                cs = slice(ci * 512, ci * 512 + 512)
                nc.sync.dma_start_transpose(
                    fT_next[:, 4 * ci : 4 * ci + 4, :], f16t[0:64, cs]
                )
                nc.sync.dma_start_transpose(
                    fT_next[:, 8 + 4 * ci : 8 + 4 * ci + 4, :], f16t[64:128, cs]
                )

            def fT_slice(fT, k):
                return fT[:, k, :]

            nt_tiles = {}

            def nt_prefetch(st, engine=None):
                if st < n_st:
                    ntt = ntp.tile([128, KT, 128], f8, tag="nt")
                    eng = engine if engine is not None else nc.gpsimd
                    eng.dma_start(ntt[:], nt_v[:, :, st * 128 : (st + 1) * 128])
                    nt_tiles[st] = ntt

            # startup loads split across both HWDGE queues: sync takes x0 +
            # the bt chunks the first scan groups touch; scalar takes noise
            nt_prefetch(0, engine=nc.scalar)
            nt_prefetch(1, engine=nc.scalar)
            nc.sync.dma_start(x[:], x0_d[:])
            nc.scalar.dma_start(cb[:], cb_d[:])
            # scan chunk c0's col-tiled pair reads bt cols 0-512 and 1024-1536
            for n in (0, 2):
                nc.sync.dma_start(
                    bt[:, :, n * 512 : (n + 1) * 512],
                    bt_v[:, :, n * 512 : (n + 1) * 512],
                )
            for n in range(4):
                nc.scalar.dma_start(
                    gt[:, :, n * 512 : (n + 1) * 512],
                    gt_v[:, :, n * 512 : (n + 1) * 512],
                )
            for n in (1, 3):
                nc.sync.dma_start(
                    bt[:, :, n * 512 : (n + 1) * 512],
                    bt_v[:, :, n * 512 : (n + 1) * 512],
                )

            # ---- initial f(x0) ----
            fT_cur = fTp.tile([128, KT, 64], f16, tag="fT")
            f16t0 = f16p.tile([128, HALF], f16, tag="f16t")
            for ci, (c0, c1) in enumerate(CHUNKS):
                nl_chunk(x[:, c0:c1], f16t0[:, c0:c1], c1 - c0)
                transposes_chunk(f16t0, fT_cur, ci)

            zf_tiles = {}
            zu_tiles = {}


            def noise_mm(st, half):
                """Matmuls for half a noise supertile (issued mid-step so the
                PE stays busy while the scan epilogue runs). Copy-out is
                issued separately (noise_copy) to avoid DVE head-of-line
                blocking of the chain ops."""
                if half == 0:
                    ntt = nt_tiles.pop(st)
                    zu = zup.tile([128, SIZE], f16, tag="zu")
                    zu_tiles[st] = (ntt, zu)
                    nt_prefetch(st + 1)
                else:
                    ntt, zu = zu_tiles[st]
                pss = []
                for n in (2 * half, 2 * half + 1):
                    ps = npsp.tile([128, 512], f32, tag="nps")
                    for k in range(KT // 2):
                        nc.tensor.matmul(
                            ps[:],
                            ntt[:, 2 * k : 2 * k + 2, :],
                            gt[:, 2 * k : 2 * k + 2, n * 512 : (n + 1) * 512],
                            start=(k == 0),
                            stop=(k == KT // 2 - 1),
                            perf_mode=PM.DoubleRow,
                        )
                    pss.append((n, ps))
                return pss

            def noise_copy(st, half, pss):
                _, zu = zu_tiles[st]
                for n, ps in pss:
                    # scaled copy out (fp32 psum -> fp16 sbuf): undo fp8 input
                    # scales, apply sqrt_dt*eps/(1-dt), add colsum broadcast
                    ncol = slice(n * 512, (n + 1) * 512)
                    nc.vector.scalar_tensor_tensor(
                        zu[:, ncol], ps[:], NOISE_OUT_SCALE, cb[:, ncol], OP.mult, OP.add
                    )
                if half == 1:
                    zu_tiles.pop(st)
                    # fold into per-step layout via SBUF->SBUF DMA on gpsimd
                    # (multi-step slack; keeps both HWDGE queues clear)
                    for h in (0, 1):
                        t = 2 * st + h
                        zf = zfp.tile([128, HALF], f16, tag="zf")
                        nc.gpsimd.dma_start(zf[0:64, :], zu[h * 64 : h * 64 + 64, 0:HALF])
                        nc.gpsimd.dma_start(
                            zf[64:128, :], zu[h * 64 : h * 64 + 64, HALF:SIZE]
                        )
                        zf_tiles[t] = zf

            def noise_half(st, half):
                noise_copy(st, half, noise_mm(st, half))

            # lead: two supertiles (z for steps 0-3) before the scan
            for st0 in range(min(2, n_st)):
                noise_half(st0, 0)
                noise_half(st0, 1)

            # ---- the scan ----
            # PE program per step:
            #   [c0 k-tiles from prev c0][c1 same][noise block][c0 k-tiles
            #   from prev c1 + epilogue][c1 same]
            # The noise block sits where the PE would otherwise stall waiting
            # for the previous step's chunk-1 epilogue + transposes.
            K_EARLY = [0, 1, 2, 3, 8, 9, 10, 11]  # produced by prev chunk 0
            K_LATE = [4, 5, 6, 7, 12, 13, 14, 15]  # produced by prev chunk 1

            def scan_group(ps, ks, ci, start):
                c0, c1 = CHUNKS[ci]
                for i, k in enumerate(ks):
                    fTk = fT_slice(fT_cur, k)
                    st_flag = start and i == 0
                    sp_flag = (not start) and i == len(ks) - 1
                    nc.tensor.matmul(
                        ps[0:64, :],
                        fTk,
                        bt[:, k, c0:c1],
                        start=st_flag,
                        stop=sp_flag,
                        tile_position=(0, 0),
                    )
                    nc.tensor.matmul(
                        ps[64:128, :],
                        fTk,
                        bt[:, k, HALF + c0 : HALF + c1],
                        start=st_flag,
                        stop=sp_flag,
                        tile_position=(0, 64),
                    )

            for t in range(steps):
                zf = zf_tiles.pop(t)
                # w' = x + (z + c)/A_COEF   (gpsimd: off the DVE critical path)
                nc.gpsimd.tensor_add(w[:], x[:], zf[:])
                fT_next = fTp.tile([128, KT, 64], f16, tag="fT")
                f16t = f16p.tile([128, HALF], f16, tag="f16t")
                ps0 = spsp.tile([128, 512], f32, tag="sps0")
                ps1 = spsp.tile([128, 512], f32, tag="sps1")
                ps_c = [ps0, ps1]
                for ci in (0, 1):
                    scan_group(ps_c[ci], K_EARLY, ci, start=True)
                # noise matmuls fill the PE while the previous step's chunk-1
                # epilogue lands; their copy-outs are issued at end of step so
                # they can't head-of-line-block the chain ops on the DVE
                st, half = t // 2 + 2, t % 2
                pss = noise_mm(st, half) if st < n_st else None
                for ci, (c0, c1) in enumerate(CHUNKS):
                    scan_group(ps_c[ci], K_LATE, ci, start=False)
                    # x_new = A_COEF*w' + psum
                    nc.vector.scalar_tensor_tensor(
                        x[:, c0:c1], w[:, c0:c1], A_COEF, ps_c[ci][:], OP.mult, OP.add
                    )
                    nl_chunk(x[:, c0:c1], f16t[:, c0:c1], c1 - c0)
                    transposes_chunk(f16t, fT_next, ci)
                if pss is not None:
                    noise_copy(st, half, pss)
                fT_cur = fT_next

            nc.scalar.dma_start(out_d[:], x[:])

    nc.compile()
    return nc


def _prepare_host(x, A, G, noise, steps):
    """Host-side O(n^2) weight prep + per-core input shards."""
    import concourse.mybir as mybir

    f8np = mybir.dt.np(mybir.dt.float8e4)
    block = SIZE // N_CLASSES
    P = np.zeros((SIZE, SIZE), dtype=np.float32)
    for c in range(N_CLASSES):
        P[c * block : (c + 1) * block, c * block : (c + 1) * block] = 1.0 / block
    Ab = A.reshape(SIZE, N_CLASSES, block).mean(axis=2)
    A_P = np.repeat(Ab, block, axis=1)  # A @ P
    M0 = A - A_P + EIG * P  # A @ (I-P) + P
    Bp = (DT * BETA) * M0.T  # B' with dt*beta folded
    bt_np = np.ascontiguousarray((-GAMMA * R_SC) * Bp).astype(np.float16)
    # colsum(B')/A_COEF, broadcast: added to z at copy-out (z also /A_COEF)
    c_np = (Bp.sum(axis=0, dtype=np.float64) / A_COEF).astype(np.float32)
    cb_np = np.ascontiguousarray(np.broadcast_to(c_np, (128, SIZE)), dtype=np.float32)
    gt_np = np.ascontiguousarray(
        np.clip(G.T * SG_SCALE, -240.0, 240.0).astype(f8np)
    )

    in_maps = []
    for c in range(N_CORES):
        xs = x[c * BPC : (c + 1) * BPC]
        x0f = np.concatenate([xs[:, :HALF], xs[:, HALF:]], axis=0)
        x0f = np.ascontiguousarray(x0f, dtype=np.float32)
        nsh = noise[:steps, c * BPC : (c + 1) * BPC, :].reshape(steps * BPC, SIZE)
        nt_np = np.ascontiguousarray(
            np.clip(nsh.T * SN_SCALE, -240.0, 240.0).astype(f8np)
        )
        in_maps.append(
            {"x0": x0f, "bt": bt_np, "gt": gt_np, "nt": nt_np, "cb": cb_np}
        )
    return in_maps


def _run(in_maps, steps, trace=False):
    from concourse.bass_utils import run_bass_kernel_spmd

    key = steps
    if key not in _cache:
        _cache[key] = _build(steps)
    nc = _cache[key]
    res = run_bass_kernel_spmd(nc, in_maps, list(range(N_CORES)), trace=trace)
    outs = []
    for c in range(N_CORES):
        of = res.results[c]["out"]
        outs.append(np.concatenate([of[0:64, :], of[64:128, :]], axis=1))
    return np.concatenate(outs, axis=0).astype(np.float32), res


def kernel(x, A, G, noise):
    x = np.asarray(x, dtype=np.float32)
    A = np.asarray(A, dtype=np.float32)
    G = np.asarray(G, dtype=np.float32)
    noise = np.asarray(noise, dtype=np.float32)
    in_maps = _prepare_host(x, A, G, noise, STEPS)
    out, _ = _run(in_maps, STEPS)
    return out

